# revision 16
# baseline (speedup 1.0000x reference)
"""Trainium2 Bass kernel for nn_Block2DGRU (norm->dwconv3x3->bi-minGRU->norm->MLP).

v2: feature-on-partitions layout; fp8-e4m3 DoubleRow matmuls for the GRU
gate projection (weights x512, activations unscaled), bf16 for every other
matmul (conv taps, wout, MLP, LN stats/broadcasts); elementwise work spread
across Act/DVE/Pool (Pool cannot touch PSUM); bias terms preloaded into
PSUM via K=1 matmuls so PSUM consumers are single fused ops; weights
resident in SBUF across both batch items.  SPMD over 8 NeuronCores,
data-parallel over batch (2 items per core).
"""
import numpy as np
import ml_dtypes

import concourse.bass as bass
import concourse.tile as tile
import concourse.mybir as mybir
from concourse.bass_utils import run_bass_kernel_spmd

F32 = mybir.dt.float32
BF16 = mybir.dt.bfloat16
F8E4 = mybir.dt.float8e4
AF = mybir.ActivationFunctionType
ALU = mybir.AluOpType
DRM = mybir.MatmulPerfMode.DoubleRow

E4NP = ml_dtypes.float8_e4m3fn
BFNP = ml_dtypes.bfloat16

NB = 56
L = NB * NB            # 3136
D = 384
DC = 3
DI = 768               # gru inner
DIC = 6
HG = 2 * DI            # 1536 (hidden|gate)
MLP = 1536
MLPC = 12
B = 2                  # batch per core
NCORES = 8
NT1 = 448              # LN/MLP block (L = 7*448)
NB1 = 7
NTC = 392              # conv slab (7 image rows)
NSLAB = 8
QT = 784               # gru quarter
NQ = 4
NTG = 392              # gru matmul half
EPS = 1e-5
SW = 512.0             # fp8 weight scale
HB = 256.0             # psum-preloaded bias: SW * 0.5


# ---------------------------------------------------------------- wait fix
def _fix_multiwaits(nc):
    """The walrus accepts at most ONE sync wait per instruction; hoist
    extras into wait-only NoOps on the same engine (streams are in-order)."""
    cnt = [0]
    for f in nc.m.functions:
        for bb in f.blocks:
            out = []
            for inst in bb.instructions:
                si = inst.sync_info
                if si is not None and si.on_wait is not None and len(si.on_wait) > 1:
                    waits = list(si.on_wait)
                    for w in waits[:-1]:
                        cnt[0] += 1
                        nop = mybir.InstNoOp(
                            name=f"I-waitfix-{cnt[0]}",
                            sync_info=mybir.SyncInfo(on_wait=[w], on_update=[]),
                        )
                        nop.engine = inst.engine
                        out.append(nop)
                    inst.sync_info = mybir.SyncInfo(
                        on_wait=[waits[-1]], on_update=list(si.on_update or [])
                    )
                out.append(inst)
            bb.instructions = out
    return cnt[0]


def _conv_tap_ranges(tap, slab):
    """valid out rows [r0, r1) within image and cols [c0, c1) for tap."""
    dr, dc = tap // 3 - 1, tap % 3 - 1
    rlo, rhi = max(0, -dr), min(NB - 1, NB - 1 - dr)
    r0 = max(7 * slab, rlo)
    r1 = min(7 * slab + 6, rhi)
    c0, c1 = max(0, -dc), min(NB - 1, NB - 1 - dc)
    return dr, dc, r0, r1 + 1, c0, c1 + 1


# ---------------------------------------------------------------- builder
def build_kernel():
    nc = bass.Bass("TRN2", target_bir_lowering=False, debug=False,
                   num_devices=NCORES)

    xT_d = nc.dram_tensor("xT", [B, D, L], BF16, kind="ExternalInput").ap()
    whgp_d = [nc.dram_tensor(f"whgp{g}", [128, 2 * HG], F8E4,
                             kind="ExternalInput").ap() for g in range(2)]
    whgs_d = [nc.dram_tensor(f"whgs{g}", [128, HG], F8E4,
                             kind="ExternalInput").ap() for g in range(2)]
    wout_d = [nc.dram_tensor(f"wout{g}", [128, DIC * D], BF16,
                             kind="ExternalInput").ap() for g in range(2)]
    p1p_d = nc.dram_tensor("p1p", [128, 2 * MLP], BF16, kind="ExternalInput").ap()
    p1s_d = nc.dram_tensor("p1s", [128, MLP], BF16, kind="ExternalInput").ap()
    p2_d = nc.dram_tensor("p2", [128, MLPC * D], BF16, kind="ExternalInput").ap()
    dgw_d = nc.dram_tensor("dgw", [DC, 128, 9 * 128], BF16,
                           kind="ExternalInput").ap()
    dwb_d = nc.dram_tensor("dwb", [128, DC], F32, kind="ExternalInput").ap()
    p1b_d = nc.dram_tensor("p1br", [1, MLP], BF16, kind="ExternalInput").ap()
    p2b_d = nc.dram_tensor("p2br", [1, D], BF16, kind="ExternalInput").ap()
    out_d = nc.dram_tensor("outT", [B, D, L], F32, kind="ExternalOutput").ap()

    from contextlib import ExitStack
    with tile.TileContext(nc) as tc, ExitStack() as ctx:
        wpool = ctx.enter_context(tc.tile_pool(name="wpool", bufs=1))
        big = ctx.enter_context(tc.tile_pool(name="big", bufs=1))
        work = ctx.enter_context(tc.tile_pool(name="work", bufs=2))
        psum = ctx.enter_context(tc.tile_pool(name="psum", bufs=1, space="PSUM"))

        def ptile(tag, shape=None):
            sz = {"pA": [128, 2048], "pB": [128, 1024], "pC": [128, 1024]}[tag]
            return psum.tile(sz, F32, tag=tag, name=tag, bufs=1)

        # ---- persistent constants / weights (loaded once)
        ones_f = wpool.tile([128, 1], F32, tag="ones_f", name="ones_f")
        nc.vector.memset(ones_f[:], 1.0)
        ones_col = wpool.tile([128, 1], BF16, tag="ones_col", name="ones_col")
        nc.vector.tensor_copy(ones_col[:], ones_f[:])
        ones_row = wpool.tile([1, 128], BF16, tag="ones_row", name="ones_row")
        nc.vector.memset(ones_row[:], 1.0)
        ones448 = wpool.tile([1, NT1], BF16, tag="ones448", name="ones448")
        nc.vector.memset(ones448[:], 1.0)
        row256 = wpool.tile([1, NTG], BF16, tag="row256", name="row256")
        nc.vector.memset(row256[:], HB)
        eps_t = wpool.tile([1, 1], F32, tag="eps", name="eps")
        nc.vector.memset(eps_t[:], EPS)
        negh = wpool.tile([128, 1], F32, tag="negh", name="negh")
        nc.vector.memset(negh[:], -0.5)

        whgp = [wpool.tile([128, 2 * HG], F8E4, tag=f"whgp{g}", name=f"whgp{g}")
                for g in range(2)]
        whgs = [wpool.tile([128, HG], F8E4, tag=f"whgs{g}", name=f"whgs{g}")
                for g in range(2)]
        wout = [wpool.tile([128, DIC * D], BF16, tag=f"wout{g}", name=f"wout{g}")
                for g in range(2)]
        for g in range(2):
            nc.sync.dma_start(whgp[g][:], whgp_d[g])
            nc.sync.dma_start(whgs[g][:], whgs_d[g])
            nc.sync.dma_start(wout[g][:], wout_d[g])
        p1p = wpool.tile([128, 2 * MLP], BF16, tag="p1p", name="p1p")
        nc.sync.dma_start(p1p[:], p1p_d)
        p1s = wpool.tile([128, MLP], BF16, tag="p1s", name="p1s")
        nc.sync.dma_start(p1s[:], p1s_d)
        p2w = wpool.tile([128, MLPC * D], BF16, tag="p2w", name="p2w")
        nc.sync.dma_start(p2w[:], p2_d)
        dgw = [wpool.tile([128, 9 * 128], BF16, tag=f"dgw{c}", name=f"dgw{c}")
               for c in range(DC)]
        for c in range(DC):
            nc.sync.dma_start(dgw[c][:], dgw_d[c])
        dwb = wpool.tile([128, DC], F32, tag="dwb", name="dwb")
        nc.sync.dma_start(dwb[:], dwb_d)
        p1br = wpool.tile([1, MLP], BF16, tag="p1br", name="p1br")
        nc.sync.dma_start(p1br[:], p1b_d)
        p2br = wpool.tile([1, D], BF16, tag="p2br", name="p2br")
        nc.sync.dma_start(p2br[:], p2b_d)

        whgpv = [whgp[g][:].rearrange("p (two m) -> p two m", two=2)
                 for g in range(2)]
        woutv = [wout[g][:].rearrange("p (k m) -> p k m", m=D)
                 for g in range(2)]
        p1pv = p1p[:].rearrange("p (two m) -> p two m", two=2)
        p2v = p2w[:].rearrange("p (k m) -> p k m", m=D)

        for b in range(B):
            # ---- input (bf16, host-transposed/cast)
            xb = [big.tile([128, L], BF16, tag=f"xb{c}", name=f"xb{c}")
                  for c in range(DC)]
            for c in range(DC):
                nc.sync.dma_start(xb[c][:], xT_d[b, c * 128:(c + 1) * 128, :])

            # ---- shared layernorm
            def layer_norm(src, dst_write):
                mmrows = work.tile([1, 2 * L], BF16, tag="mmrows",
                                   name="mmrows", bufs=1)
                for blk in range(NB1):
                    sl = slice(blk * NT1, (blk + 1) * NT1)
                    pb = ptile("pB")
                    s_ps = pb[0:1, 0:NT1]
                    q_ps = pb[0:1, 512:512 + NT1]
                    for c in range(DC):
                        sq = work.tile([128, NT1], BF16, tag="sq", name="sq",
                                       bufs=2)
                        nc.gpsimd.tensor_tensor(sq[:], src[c][:, sl],
                                                src[c][:, sl], ALU.mult)
                        nc.tensor.matmul(s_ps, ones_col[:], src[c][:, sl],
                                         start=(c == 0), stop=(c == DC - 1))
                        nc.tensor.matmul(q_ps, ones_col[:], sq[:],
                                         start=(c == 0), stop=(c == DC - 1))
                    st = work.tile([97, NT1], F32, tag="strow", name="strow",
                                   bufs=2)
                    nc.scalar.activation(mmrows[0:1, sl], s_ps,
                                         AF.Copy, scale=-1.0 / D)
                    nc.scalar.activation(st[0:1, :], s_ps,
                                         AF.Square, scale=1.0 / D)
                    nc.vector.scalar_tensor_tensor(st[32:33, :], q_ps,
                                                   1.0 / D, st[0:1, :],
                                                   ALU.mult, ALU.subtract)
                    nc.scalar.activation(st[64:65, :], st[32:33, :], AF.Sqrt,
                                         bias=eps_t[:])
                    # reciprocal over [7,64] (DMA reshape) — [1,448] is serial
                    pk = work.tile([7, 64], F32, tag="pk", name="pk", bufs=2)
                    nc.sync.dma_start(pk[:], st[64:65, :])
                    ik = work.tile([7, 64], BF16, tag="ik", name="ik", bufs=2)
                    with nc.allow_low_precision(reason="bf16 inv row"):
                        nc.vector.reciprocal(ik[:], pk[:])
                    nc.sync.dma_start(mmrows[0:1, L + blk * NT1:
                                             L + (blk + 1) * NT1], ik[:])
                for blk in range(NB1):
                    sl = slice(blk * NT1, (blk + 1) * NT1)
                    bc = ptile("pC")
                    nc.tensor.matmul(bc[:, 0:NT1], ones_row[:],
                                     mmrows[0:1, sl], start=True, stop=True)
                    nc.tensor.matmul(bc[:, 512:512 + NT1], ones_row[:],
                                     mmrows[0:1, L + blk * NT1:
                                            L + (blk + 1) * NT1],
                                     start=True, stop=True)
                    # Pool can't read PSUM: stage rows to SBUF once (Act)
                    mbib = work.tile([128, 2 * NT1], BF16, tag="mbib",
                                     name="mbib", bufs=2)
                    mbibv = mbib[:].rearrange("p (two n) -> p two n", two=2)
                    bcv = bc[:].rearrange("p (two n) -> p two n",
                                          two=2)[:, :, 0:NT1]
                    nc.scalar.activation(mbibv, bcv, AF.Copy)
                    for c in range(DC):
                        t = work.tile([128, NT1], BF16, tag="tap", name="tap",
                                      bufs=3)
                        nc.gpsimd.tensor_tensor(t[:], src[c][:, sl],
                                                mbib[:, 0:NT1], ALU.add)
                        dst_write(c, sl, t, mbib[:, NT1:2 * NT1])

            # ============================== phase N1
            xh = [big.tile([128, L], BF16, tag=f"xh{c}", name=f"xh{c}")
                  for c in range(DC)]

            def write_xh(c, sl, t, ib):
                nc.gpsimd.tensor_tensor(xh[c][:, sl], t[:], ib, ALU.mult)

            layer_norm(xb, write_xh)

            # ============================== phase C: dw conv 3x3 (bf16)
            hc8 = big.tile([128, DC * L], F8E4, tag="hc8", name="hc8")
            hc8v = hc8[:].rearrange("p (c t) -> p c t", t=L)
            tap_order = [0, 3, 6, 2, 5, 8, 1, 4, 7]
            for c in range(DC):
                for slab in range(NSLAB):
                    w0 = max(0, 7 * slab - 1)
                    w1 = min(NB, 7 * slab + 8)
                    nw = w1 - w0
                    win = xh[c][:, w0 * NB:w1 * NB]
                    xm = work.tile([128, 512], BF16, tag="xm", name="xm",
                                   bufs=2)
                    nc.gpsimd.tensor_copy(xm[:, 1:nw * NB], win[:, 0:nw * NB - 1])
                    xm3 = xm[:, 0:nw * NB].rearrange("p (r cc) -> p r cc", cc=NB)
                    nc.gpsimd.memset(xm3[:, :, 0:1], 0.0)
                    xp = work.tile([128, 512], BF16, tag="xp", name="xp",
                                   bufs=2)
                    nc.gpsimd.tensor_copy(xp[:, 0:nw * NB - 1], win[:, 1:nw * NB])
                    xp3 = xp[:, 0:nw * NB].rearrange("p (r cc) -> p r cc", cc=NB)
                    nc.gpsimd.memset(xp3[:, :, NB - 1:NB], 0.0)
                    cpb = ptile("pB")
                    cp = cpb[:, 0:NTC] if slab % 2 == 0 else cpb[:, 512:512 + NTC]
                    for ti, tap in enumerate(tap_order):
                        dr, dcc, r0, r1, c0, c1 = _conv_tap_ranges(tap, slab)
                        osl = slice((r0 - 7 * slab) * NB, (r1 - 7 * slab) * NB)
                        if dcc == 0:
                            rhs = xh[c][:, (r0 + dr) * NB:(r1 + dr) * NB]
                        elif dcc == -1:
                            rhs = xm[:, (r0 + dr - w0) * NB:(r1 + dr - w0) * NB]
                        else:
                            rhs = xp[:, (r0 + dr - w0) * NB:(r1 + dr - w0) * NB]
                        nc.tensor.matmul(
                            cp[:, osl], dgw[c][:, tap * 128:(tap + 1) * 128],
                            rhs, start=(ti == 0), stop=(ti == 8))
                    nc.vector.tensor_scalar(
                        hc8v[:, c, slab * NTC:(slab + 1) * NTC], cp[:],
                        dwb[:, c:c + 1], None, ALU.add)

            # ============================== phase G: bi-minGRU
            y1 = [big.tile([128, L], BF16, tag=f"y{c}", name=f"y{c}")
                  for c in range(DC)]
            y2 = [big.tile([128, L], BF16, tag=f"xh{c}", name=f"y2{c}")
                  for c in range(DC)]
            hs = [work.tile([128, DIC * QT], BF16, tag=f"hs{g}",
                            name=f"hs{g}", bufs=1) for g in range(2)]
            hsv = [hs[g][:].rearrange("p (j t) -> p j t", t=QT)
                   for g in range(2)]
            carry = [work.tile([128, DIC], F32, tag=f"carry{g}",
                               name=f"carry{g}", bufs=1) for g in range(2)]

            def gru_hg_scan(g, q, qi):
                qsl0 = q * QT
                for j in range(DIC):
                    # hidden in pA quarters 0-1 (+HB preload), gate in 2-3
                    # (+HB preload) so one Act op sigmoids all four with the
                    # same scale/bias
                    pa = ptile("pA")
                    for half, coff in ((0, 0), (1, DI)):
                        for nh in range(2):
                            nsl = slice(qsl0 + nh * NTG, qsl0 + (nh + 1) * NTG)
                            dst = pa[:, (2 * half + nh) * 512:
                                     (2 * half + nh) * 512 + NTG]
                            nc.tensor.matmul(dst, ones_row[:], row256[:],
                                             start=True, stop=False)
                            nc.tensor.matmul(
                                dst,
                                whgpv[g][:, :, coff + j * 128:
                                         coff + (j + 1) * 128],
                                hc8v[:, 0:2, nsl], start=False, stop=False,
                                perf_mode=DRM)
                            nc.tensor.matmul(
                                dst,
                                whgs[g][:, coff + j * 128:coff + (j + 1) * 128],
                                hc8v[:, 2, nsl], start=False, stop=True)
                    pav = pa[:].rearrange("p (q n) -> p q n", q=4)[:, :, 0:NTG]
                    # zs = [s (hidden sigmoid) | z (gate sigmoid)]
                    zs = work.tile([128, 4 * NTG], BF16, tag="zs", name="zs",
                                   bufs=2)
                    zsv = zs[:].rearrange("p (q n) -> p q n", q=4)
                    nc.scalar.activation(zsv, pav, AF.Sigmoid,
                                         scale=1.0 / SW, bias=negh[:])
                    s_ap = zs[:, 0:QT]
                    z_ap = zs[:, QT:2 * QT]
                    # g~ = max(h + 0.5, s) ; pa quarters 0-1 hold SW*(h+0.5)
                    tg = work.tile([128, QT], BF16, tag="tg", name="tg",
                                   bufs=2)
                    tgv = tg[:].rearrange("p (two n) -> p two n", two=2)
                    nc.vector.scalar_tensor_tensor(
                        tgv, pav[:, 0:2, :], 1.0 / SW,
                        zsv[:, 0:2, :], ALU.mult, ALU.max)
                    a = work.tile([128, QT], BF16, tag="a", name="a", bufs=2)
                    nc.gpsimd.tensor_scalar(a[:], z_ap, -1.0, 1.0,
                                            ALU.mult, ALU.add)
                    bb = work.tile([128, QT], BF16, tag="bb", name="bb",
                                   bufs=2)
                    nc.gpsimd.tensor_tensor(bb[:], z_ap, tg[:], ALU.mult)
                    init = 0.0 if qi == 0 else carry[g][:, j:j + 1]
                    dst = hs[g][:, j * QT:(j + 1) * QT]
                    if g == 0:
                        nc.vector.tensor_tensor_scan(dst, a[:], bb[:], init,
                                                     ALU.mult, ALU.add)
                        nc.vector.tensor_copy(
                            carry[g][:, j:j + 1],
                            hs[g][:, (j + 1) * QT - 1:(j + 1) * QT])
                    else:
                        rv = slice(None, None, -1)
                        nc.vector.tensor_tensor_scan(dst[:, rv], a[:, rv],
                                                     bb[:, rv], init,
                                                     ALU.mult, ALU.add)
                        nc.vector.tensor_copy(carry[g][:, j:j + 1],
                                              hs[g][:, j * QT:j * QT + 1])

            def gru_wout(g, q):
                ytiles = y1 if g == 0 else y2
                for dc in range(DC):
                    yp = ptile("pC")
                    for nh in range(2):
                        dst = yp[:, nh * 512:nh * 512 + NTG]
                        for k in range(DIC):
                            nc.tensor.matmul(
                                dst, woutv[g][:, k, dc * 128:(dc + 1) * 128],
                                hs[g][:, k * QT + nh * NTG:
                                      k * QT + (nh + 1) * NTG],
                                start=(k == 0), stop=(k == DIC - 1))
                    ypv = yp[:].rearrange("p (two n) -> p two n",
                                          two=2)[:, :, 0:NTG]
                    odst = ytiles[dc][:, q * QT:(q + 1) * QT]
                    ov = odst.rearrange("p (two n) -> p two n", two=2)
                    nc.vector.tensor_copy(ov, ypv)

            sched = []
            for qi in range(NQ):
                sched.append((0, qi, qi))
                sched.append((1, NQ - 1 - qi, qi))
            prev = None
            for (g, q, qi) in sched:
                gru_hg_scan(g, q, qi)
                if prev is not None:
                    gru_wout(*prev)
                prev = (g, q)
            gru_wout(*prev)

            # residual merge: y = y1 + y2 + x
            for c in range(DC):
                nc.vector.tensor_tensor(y1[c][:], y1[c][:], y2[c][:], ALU.add)
                nc.gpsimd.tensor_tensor(y1[c][:], y1[c][:], xb[c][:], ALU.add)

            # ============================== phase N2
            yh8 = big.tile([128, DC * L], F8E4, tag="hc8", name="yh8")
            yh8v = yh8[:].rearrange("p (c t) -> p c t", t=L)

            def write_yh(c, sl, t, ib):
                nc.vector.tensor_tensor(yh8v[:, c, sl], t[:], ib, ALU.mult)

            layer_norm(y1, write_yh)

            # ============================== phase M: MLP (bf16)
            def mlp_p1(blk):
                sl = slice(blk * NT1, (blk + 1) * NT1)
                yst = work.tile([128, DC * NT1], BF16, tag="yst", name="yst",
                                bufs=2)
                ystv = yst[:].rearrange("p (c t) -> p c t", t=NT1)
                nc.gpsimd.tensor_copy(ystv, yh8v[:, :, sl])
                q8 = work.tile([128, MLPC * NT1], BF16, tag="q8", name="q8",
                               bufs=2)
                q8v = q8[:].rearrange("p (m t) -> p m t", t=NT1)
                for mp in range(MLPC // 2):
                    pa = ptile("pA")
                    base = (mp % 2) * 1024
                    for i in range(2):
                        mi = 2 * mp + i
                        dst = pa[:, base + i * 512:base + i * 512 + NT1]
                        nc.tensor.matmul(dst,
                                         p1br[0:1, mi * 128:(mi + 1) * 128],
                                         ones448[:], start=True, stop=False)
                        for k in range(DC):
                            lhs = (p1pv[:, k, mi * 128:(mi + 1) * 128]
                                   if k < 2
                                   else p1s[:, mi * 128:(mi + 1) * 128])
                            nc.tensor.matmul(dst, lhs, ystv[:, k, :],
                                             start=False, stop=(k == DC - 1))
                    pav = pa[:, base:base + 1024].rearrange(
                        "p (two n) -> p two n", two=2)[:, :, 0:NT1]
                    nc.scalar.activation(q8v[:, 2 * mp:2 * mp + 2, :], pav,
                                         AF.Gelu)
                return q8v

            def mlp_p2(blk, q8v):
                sl = slice(blk * NT1, (blk + 1) * NT1)
                for dc in range(DC):
                    pb = ptile("pB")
                    dst = pb[:, (dc % 2) * 512:(dc % 2) * 512 + NT1]
                    nc.tensor.matmul(dst, p2br[0:1, dc * 128:(dc + 1) * 128],
                                     ones448[:], start=True, stop=False)
                    for k in range(MLPC):
                        nc.tensor.matmul(
                            dst, p2v[:, k, dc * 128:(dc + 1) * 128],
                            q8v[:, k, :], start=False, stop=(k == MLPC - 1))
                    oo = work.tile([128, NT1], F32, tag="oo", name="oo",
                                   bufs=3)
                    nc.vector.tensor_tensor(oo[:], dst, y1[dc][:, sl],
                                            ALU.add)
                    nc.sync.dma_start(out_d[b, dc * 128:(dc + 1) * 128, sl],
                                      oo[:])

            prevq = None
            for blk in range(NB1):
                q8v = mlp_p1(blk)
                if prevq is not None:
                    mlp_p2(blk - 1, prevq)
                prevq = q8v
            mlp_p2(NB1 - 1, prevq)

    return nc


# ---------------------------------------------------------------- host side
_NC_CACHE = {}


def _get_nc():
    if "v2" not in _NC_CACHE:
        nc = build_kernel()
        _fix_multiwaits(nc)
        _NC_CACHE["v2"] = nc
    return _NC_CACHE["v2"]


def _prep_weights(inp):
    f = np.float32
    dw_w = np.asarray(inp["dw_w"], f)              # [D,1,3,3]
    norm_w = np.asarray(inp["norm_w"], f)
    norm_b = np.asarray(inp["norm_b"], f)
    dw_wf = dw_w[:, 0] * norm_w[:, None, None]     # [D,3,3]
    dw_bf = np.asarray(inp["dw_b"], f) + norm_b * dw_w[:, 0].sum(axis=(1, 2))
    p1_w = np.asarray(inp["p1_w"], f)
    p1f = p1_w * np.asarray(inp["norm2_w"], f)[:, None]
    p1bf = np.asarray(inp["p1_b"], f) + np.asarray(inp["norm2_b"], f) @ p1_w

    diag = np.zeros((DC, 128, 9 * 128), f)
    ar = np.arange(128)
    for c in range(DC):
        for tap in range(9):
            dr, dcc = tap // 3, tap % 3
            diag[c, ar, tap * 128 + ar] = dw_wf[c * 128:(c + 1) * 128, dr, dcc]

    out = {}
    for g, key in enumerate(["gru1_whg", "gru2_whg"]):
        whg = np.asarray(inp[key], f) * SW          # [384, 1536]
        out[f"whgp{g}"] = np.ascontiguousarray(
            np.stack([whg[0:128], whg[128:256]], axis=1).reshape(128, 2 * HG)
        ).astype(E4NP)
        out[f"whgs{g}"] = np.ascontiguousarray(whg[256:384]).astype(E4NP)
    for g, key in enumerate(["gru1_wout", "gru2_wout"]):
        w = np.asarray(inp[key], f)                 # [768, 384]
        out[f"wout{g}"] = np.ascontiguousarray(
            w.reshape(DIC, 128, D).transpose(1, 0, 2).reshape(128, DIC * D)
        ).astype(BFNP)
    out["p1p"] = np.ascontiguousarray(
        np.stack([p1f[0:128], p1f[128:256]], axis=1).reshape(128, 2 * MLP)
    ).astype(BFNP)
    out["p1s"] = np.ascontiguousarray(p1f[256:384]).astype(BFNP)
    p2 = np.asarray(inp["p2_w"], f)                 # [1536, 384]
    out["p2"] = np.ascontiguousarray(
        p2.reshape(MLPC, 128, D).transpose(1, 0, 2).reshape(128, MLPC * D)
    ).astype(BFNP)
    out["dgw"] = diag.astype(BFNP)
    out["dwb"] = np.ascontiguousarray(dw_bf.reshape(DC, 128).T, f)
    out["p1br"] = p1bf.reshape(1, MLP).astype(BFNP)
    out["p2br"] = np.asarray(inp["p2_b"], f).reshape(1, D).astype(BFNP)
    return out


def kernel(**inputs):
    x = np.asarray(inputs["x"], np.float32)         # [16, L, D]
    w = _prep_weights(inputs)
    nc = _get_nc()

    in_maps = []
    for core in range(NCORES):
        xc = x[core * B:(core + 1) * B]
        xT = np.ascontiguousarray(xc.transpose(0, 2, 1)).astype(BFNP)
        m = dict(w)
        m["xT"] = xT
        in_maps.append(m)

    res = run_bass_kernel_spmd(nc, in_maps, core_ids=list(range(NCORES)))
    outs = []
    for core in range(NCORES):
        oT = res.results[core]["outT"]              # [B, D, L]
        outs.append(np.asarray(oT, np.float32).transpose(0, 2, 1))
    return np.ascontiguousarray(np.concatenate(outs, axis=0), np.float32)


# revision 17
# speedup vs baseline: 1.4441x; 1.4441x over previous
"""Trainium2 Bass kernel for nn_Block2DGRU (norm->dwconv3x3->bi-minGRU->norm->MLP).

Self-contained: host-side weight folding + sharding, device kernel via
Bass/Tile, SPMD over 8 NeuronCores (data-parallel over batch: 2 per core).

Device layout: everything [feature_on_partitions, time_on_free].  The minGRU
linear recurrence h_t = a_t*h_{t-1} + b_t runs natively on the DVE via
tensor_tensor_scan (per-partition scan along the free dim); GRU2 is the same
scan with reversed access patterns.
"""
import numpy as np

import concourse.bass as bass
import concourse.tile as tile
import concourse.mybir as mybir
from concourse.bass_utils import run_bass_kernel_spmd

F32 = mybir.dt.float32
F32R = mybir.dt.float32r
AF = mybir.ActivationFunctionType
ALU = mybir.AluOpType

# dims
NB = 56
L = NB * NB            # 3136
D = 384                # dim
DC = 3                 # dim chunks of 128
DI = 768               # gru inner
DIC = 6
MLP = 1536
MLPC = 12
B = 2                  # batch per core
NCORES = 8
NT = 392               # time block (= 7 image rows)
NBLK = L // NT         # 8
QT = 784               # scan quarter (= 2 blocks)
NQ = L // QT           # 4
EPS = 1e-5

MM_DT = F32R           # matmul dtype: F32 (exact, 4 cyc/row) or F32R (1 cyc/row)


# ---------------------------------------------------------------- wait fix
def _fix_multiwaits(nc):
    """This walrus accepts at most ONE sync wait per instruction; hoist
    extras into wait-only NoOps on the same engine (streams are in-order)."""
    n = 0
    cnt = [0]
    for f in nc.m.functions:
        for bb in f.blocks:
            out = []
            for inst in bb.instructions:
                si = inst.sync_info
                if si is not None and si.on_wait is not None and len(si.on_wait) > 1:
                    waits = list(si.on_wait)
                    for w in waits[:-1]:
                        cnt[0] += 1
                        nop = mybir.InstNoOp(
                            name=f"I-waitfix-{cnt[0]}",
                            sync_info=mybir.SyncInfo(on_wait=[w], on_update=[]),
                        )
                        nop.engine = inst.engine
                        out.append(nop)
                    inst.sync_info = mybir.SyncInfo(
                        on_wait=[waits[-1]], on_update=list(si.on_update or [])
                    )
                    n += 1
                out.append(inst)
            bb.instructions = out
    return n


# ---------------------------------------------------------------- builder
def _conv_tap_ranges(tap, slab):
    """valid out rows [r0, r1) within image and cols [c0, c1) for tap."""
    dr, dc = tap // 3 - 1, tap % 3 - 1
    rlo, rhi = max(0, -dr), min(NB - 1, NB - 1 - dr)
    r0 = max(7 * slab, rlo)
    r1 = min(7 * slab + 6, rhi)
    c0, c1 = max(0, -dc), min(NB - 1, NB - 1 - dc)
    return dr, dc, r0, r1 + 1, c0, c1 + 1


def build_kernel(mm_dt=MM_DT, reps=1):
    nc = bass.Bass("TRN2", target_bir_lowering=False, debug=False,
                   num_devices=NCORES)

    xT_d = nc.dram_tensor("xT", [B, D, L], F32, kind="ExternalInput").ap()
    whg1_d = nc.dram_tensor("whg1", [D, 2 * DI], F32, kind="ExternalInput").ap()
    whg2_d = nc.dram_tensor("whg2", [D, 2 * DI], F32, kind="ExternalInput").ap()
    wout1_d = nc.dram_tensor("wout1", [DI, D], F32, kind="ExternalInput").ap()
    wout2_d = nc.dram_tensor("wout2", [DI, D], F32, kind="ExternalInput").ap()
    p1_d = nc.dram_tensor("p1", [D, MLP], F32, kind="ExternalInput").ap()
    p2_d = nc.dram_tensor("p2", [MLP, D], F32, kind="ExternalInput").ap()
    diag_d = nc.dram_tensor("diag", [DC, 128, 9 * 128], F32, kind="ExternalInput").ap()
    dwb_d = nc.dram_tensor("dwb", [128, DC], F32, kind="ExternalInput").ap()
    p1b_d = nc.dram_tensor("p1b", [128, MLPC], F32, kind="ExternalInput").ap()
    p2b_d = nc.dram_tensor("p2b", [128, DC], F32, kind="ExternalInput").ap()
    out_d = nc.dram_tensor("outT", [B, D, L], F32, kind="ExternalOutput").ap()

    f32r = mm_dt == F32R

    from contextlib import ExitStack
    with tile.TileContext(nc) as tc, ExitStack() as ctx:
        big = ctx.enter_context(tc.tile_pool(name="big", bufs=1))
        wpool = ctx.enter_context(tc.tile_pool(name="wpool", bufs=1))
        work = ctx.enter_context(tc.tile_pool(name="work", bufs=2))
        psum = ctx.enter_context(tc.tile_pool(name="psum", bufs=1, space="PSUM"))
        psumb = ctx.enter_context(tc.tile_pool(name="psumb", bufs=2, space="PSUM"))

        # ---- persistent small constants
        ones_col_f = wpool.tile([128, 1], F32, tag="ones_col_f", name="ones_col_f")
        nc.vector.memset(ones_col_f[:], 1.0)
        ones1_f = wpool.tile([1, 128], F32, tag="ones1_f", name="ones1_f")
        nc.vector.memset(ones1_f[:], 1.0)
        if f32r:
            ones_col_r = wpool.tile([128, 1], F32R, tag="ones_col_r", name="ones_col_r")
            nc.vector.tensor_copy(ones_col_r[:], ones_col_f[:])
        else:
            ones_col_r = ones_col_f
        dwb_t = wpool.tile([128, DC], F32, tag="dwb", name="dwb")
        nc.sync.dma_start(dwb_t[:], dwb_d)
        p1b_t = wpool.tile([128, MLPC], F32, tag="p1b", name="p1b")
        nc.sync.dma_start(p1b_t[:], p1b_d)
        p2b_t = wpool.tile([128, DC], F32, tag="p2b", name="p2b")
        nc.sync.dma_start(p2b_t[:], p2b_d)
        eps_t = wpool.tile([1, 1], F32, tag="eps", name="eps")
        nc.vector.memset(eps_t[:], EPS)

        def load_w(src_ap, shape, tag, bufs=None):
            """DMA a weight slice; convert to mm_dt when needed."""
            if not f32r:
                t = wpool.tile(shape, F32, tag=tag, name=tag, bufs=bufs)
                nc.sync.dma_start(t[:], src_ap)
                return t
            t = wpool.tile(shape, F32R, tag=tag, name=tag, bufs=bufs)
            for c0 in range(0, shape[1], 768):
                cw = min(768, shape[1] - c0)
                st = wpool.tile([shape[0], 768], F32, tag="wstage",
                                name="wstage", bufs=2)
                nc.sync.dma_start(st[:, 0:cw], src_ap[:, c0:c0 + cw])
                nc.vector.tensor_copy(t[:, c0:c0 + cw], st[:, 0:cw])
            return t

        for rep in range(reps):
          for b in range(B):
            # ========================================== phase N1: layernorm1
            x_t = [big.tile([128, L], F32, tag=f"bufA{c}", name=f"bufA{c}")
                   for c in range(DC)]
            ob1 = range(NBLK)
            ob2 = range(NBLK - 1, -1, -1)
            for c in range(DC):
                for blk in ob1:
                    bsl = slice(blk * NT, (blk + 1) * NT)
                    nc.sync.dma_start(x_t[c][:, bsl],
                                      xT_d[b, c * 128:(c + 1) * 128, bsl])

            # stat rows: inv at partition 0, ninv at partition 32 (matmul rhs
            # needs base partition in {0,32,64}); numu/sd/m2/ve in strows.
            mmrows = work.tile([1, 2 * L], F32, tag="mmrows", name="mmrows",
                               bufs=1)
            numu_row = mmrows[0:1, 0:L]
            inv_row = mmrows[0:1, L:2 * L]
            strows = work.tile([97, NT], F32, tag="strows", name="strows",
                               bufs=1)

            def norm_stats(src_tiles, src_dt, sq_src_f32, order=None):
                ones_c = ones_col_r if src_dt == F32R else ones_col_f
                for blk in (order or range(NBLK)):
                    sl = slice(blk * NT, (blk + 1) * NT)
                    s_ps = psum.tile([1, NT], F32, tag="pA", name="s_ps")
                    q_ps = psum.tile([1, NT], F32, tag="pB", name="q_ps")
                    for c in range(DC):
                        sq = work.tile([128, NT], src_dt, tag="sq", name="sq",
                                       bufs=1)
                        nc.scalar.activation(sq[:], sq_src_f32(c, sl), AF.Square)
                        nc.tensor.matmul(s_ps[:], ones_c[:], src_tiles[c][:, sl],
                                         start=(c == 0), stop=(c == DC - 1))
                        nc.tensor.matmul(q_ps[:], ones_c[:], sq[:],
                                         start=(c == 0), stop=(c == DC - 1))
                    nc.scalar.activation(numu_row[:, sl], s_ps[:], AF.Copy,
                                         scale=-1.0 / D)
                    m2 = strows[64:65, 0:NT]
                    nc.scalar.activation(m2, s_ps[:], AF.Square, scale=1.0 / D)
                    ve = strows[96:97, 0:NT]
                    nc.vector.scalar_tensor_tensor(ve, q_ps[:], 1.0 / D,
                                                   m2, ALU.mult, ALU.subtract)
                    sd_blk = strows[32:33, 0:NT]
                    nc.scalar.activation(sd_blk, ve, AF.Sqrt, bias=eps_t[:])
                    pkb = work.tile([7, NB], F32, tag="pk", name="pkb", bufs=2)
                    nc.sync.dma_start(pkb[:], sd_blk)
                    ikb = work.tile([7, NB], F32, tag="ipk", name="ikb", bufs=2)
                    nc.vector.reciprocal(ikb[:], pkb[:])
                    nc.sync.dma_start(inv_row[:, sl], ikb[:])

            def norm_apply(dst_tiles, src_f32, order=None):
                """dst = (x + (-mu)) * inv, both rows broadcast via K=1 mms."""
                for blk in (order or range(NBLK)):
                    sl = slice(blk * NT, (blk + 1) * NT)
                    mb_ps = psum.tile([128, NT], F32,
                                      tag=("pC" if blk % 2 == 0 else "pA"),
                                      name="mb_ps")
                    nc.tensor.matmul(mb_ps[:], ones1_f[:], numu_row[:, sl],
                                     start=True, stop=True)
                    ib_ps = psum.tile([128, NT], F32,
                                      tag=("pD" if blk % 2 == 0 else "pB"),
                                      name="ib_ps")
                    nc.tensor.matmul(ib_ps[:], ones1_f[:], inv_row[:, sl],
                                     start=True, stop=True)
                    for c in range(DC):
                        t = work.tile([128, NT], F32, tag="t_ap", name="t_ap", bufs=3)
                        nc.vector.tensor_tensor(t[:], src_f32(c, sl), mb_ps[:],
                                                ALU.add)
                        nc.vector.tensor_tensor(dst_tiles[c][:, sl], t[:],
                                                ib_ps[:], ALU.mult)

            norm_stats(x_t, F32, lambda c, sl: x_t[c][:, sl], order=ob1)
            xh_t = [big.tile([128, L], mm_dt, tag=f"bufB{c}", name=f"bufB{c}")
                    for c in range(DC)]
            norm_apply(xh_t, lambda c, sl: x_t[c][:, sl], order=ob1)

            # ========================================== phase C: dw conv 3x3
            # column shifts via pre-shifted copies so every tap is a
            # row-contiguous 2D slice (f32r matmul alignment rules)
            hc_t = [big.tile([128, L], mm_dt, tag=f"bufA{c}", name=f"hc{c}")
                    for c in range(DC)]
            for c in range(DC):
                dg = load_w(diag_d[c], [128, 9 * 128], "dgw", bufs=1)
                for slab in ob1:
                    w0 = max(0, 7 * slab - 1)
                    w1 = min(NB, 7 * slab + 8)
                    nw = w1 - w0
                    win = xh_t[c][:, w0 * NB:w1 * NB].bitcast(F32)
                    xm = work.tile([128, 512], mm_dt, tag="xsh0", name="xm",
                                   bufs=1)
                    nc.vector.tensor_copy(xm[:, 1:nw * NB], win[:, 0:nw * NB - 1])
                    xm3 = xm[:, 0:nw * NB].rearrange("p (r cc) -> p r cc", cc=NB)
                    nc.vector.tensor_scalar(xm3[:, :, 0:1], xm3[:, :, 0:1],
                                            0.0, None, ALU.mult)
                    xp = work.tile([128, 512], mm_dt, tag="xsh1", name="xp",
                                   bufs=1)
                    nc.vector.tensor_copy(xp[:, 0:nw * NB - 1], win[:, 1:nw * NB])
                    xp3 = xp[:, 0:nw * NB].rearrange("p (r cc) -> p r cc", cc=NB)
                    nc.vector.tensor_scalar(xp3[:, :, NB - 1:NB],
                                            xp3[:, :, NB - 1:NB],
                                            0.0, None, ALU.mult)
                    cp = psumb.tile([128, NT], F32,
                                    tag=("hp_ps" if slab % 2 == 0 else "gp_ps"),
                                    name="conv_ps")
                    tap_order = [0, 3, 6, 2, 5, 8, 1, 4, 7]
                    for ti, tap in enumerate(tap_order):
                        dr, dcc, r0, r1, c0, c1 = _conv_tap_ranges(tap, slab)
                        nrow = r1 - r0
                        osl = slice((r0 - 7 * slab) * NB, (r1 - 7 * slab) * NB)
                        if dcc == 0:
                            rhs = xh_t[c][:, (r0 + dr) * NB:(r1 + dr) * NB]
                        elif dcc == -1:
                            rhs = xm[:, (r0 + dr - w0) * NB:(r1 + dr - w0) * NB]
                        else:
                            rhs = xp[:, (r0 + dr - w0) * NB:(r1 + dr - w0) * NB]
                        nc.tensor.matmul(
                            cp[:, osl], dg[:, tap * 128:(tap + 1) * 128], rhs,
                            start=(ti == 0), stop=(ti == 8))
                    nc.scalar.activation(
                        hc_t[c][:, slab * NT:(slab + 1) * NT], cp[:],
                        AF.Identity, bias=dwb_t[:, c:c + 1])

            # ========================================== phase G: bi-minGRU
            y_t = [big.tile([128, L], mm_dt, tag=f"bufB{c}", name=f"y{c}")
                   for c in range(DC)]
            carry = [work.tile([128, DIC], F32, tag=f"carry{g}",
                               name=f"carry{g}") for g in range(2)]

            for gi, g in enumerate((0, 1)):
                whg = [load_w((whg1_d if g == 0 else whg2_d)[k * 128:(k + 1) * 128, :],
                              [128, 2 * DI], f"whg{k}") for k in range(DC)]
                wout = [load_w((wout1_d if g == 0 else wout2_d)[k * 128:(k + 1) * 128, :],
                               [128, D], f"wout{k}", bufs=2) for k in range(DIC)]
                qorder = range(NQ) if g == 0 else range(NQ - 1, -1, -1)
                for qi, q in enumerate(qorder):
                    hs = [work.tile([128, QT], mm_dt, tag=f"hs{j}",
                                    name=f"hs{j}", bufs=1) for j in range(DIC)]
                    for j in range(DIC):
                        z = work.tile([128, QT], F32, tag="z", name="z", bufs=2)
                        s = work.tile([128, QT], F32, tag="s", name="s", bufs=2)
                        for nb2 in range(2):
                            nsl = slice(q * QT + nb2 * NT,
                                        q * QT + (nb2 + 1) * NT)
                            hsl = slice(nb2 * NT, (nb2 + 1) * NT)
                            hp = psumb.tile([128, NT], F32, tag="hp_ps",
                                            name="hp_ps")
                            gp = psumb.tile([128, NT], F32, tag="gp_ps",
                                            name="gp_ps")
                            for k in range(DC):
                                nc.tensor.matmul(
                                    hp[:], whg[k][:, j * 128:(j + 1) * 128],
                                    hc_t[k][:, nsl],
                                    start=(k == 0), stop=(k == DC - 1))
                            for k in range(DC):
                                nc.tensor.matmul(
                                    gp[:],
                                    whg[k][:, DI + j * 128:DI + (j + 1) * 128],
                                    hc_t[k][:, nsl],
                                    start=(k == 0), stop=(k == DC - 1))
                            nc.scalar.activation(z[:, hsl], gp[:], AF.Sigmoid)
                            nc.scalar.activation(s[:, hsl], hp[:], AF.Sigmoid)
                            # g = max(hidden+0.5, sigmoid(hidden)) in place
                            nc.vector.scalar_tensor_tensor(
                                s[:, hsl], hp[:], 0.5, s[:, hsl],
                                ALU.add, ALU.max)
                        bb = work.tile([128, QT], F32, tag="bb", name="bb",
                                       bufs=2)
                        nc.vector.tensor_tensor(bb[:], z[:], s[:], ALU.mult)
                        # a = 1 - z in place on z (after bb consumed z)
                        nc.scalar.activation(z[:], z[:], AF.Copy,
                                             bias=1.0, scale=-1.0)
                        init = 0.0 if qi == 0 else carry[g][:, j:j + 1]
                        if g == 0:
                            nc.vector.tensor_tensor_scan(
                                hs[j][:], z[:], bb[:], init, ALU.mult, ALU.add)
                            nc.gpsimd.tensor_copy(carry[g][:, j:j + 1],
                                                  hs[j][:, QT - 1:QT])
                        else:
                            rv = slice(None, None, -1)
                            nc.vector.tensor_tensor_scan(
                                hs[j][:, rv], z[:, rv], bb[:, rv], init,
                                ALU.mult, ALU.add)
                            nc.gpsimd.tensor_copy(carry[g][:, j:j + 1],
                                                  hs[j][:, 0:1])
                    for dc in range(DC):
                        for nb2 in range(2):
                            y_ps = psum.tile(
                                [128, NT], F32,
                                tag=("pC" if (dc * 2 + nb2) % 2 == 0 else "pD"),
                                name="y_ps")
                            for k in range(DIC):
                                nc.tensor.matmul(
                                    y_ps[:], wout[k][:, dc * 128:(dc + 1) * 128],
                                    hs[k][:, nb2 * NT:(nb2 + 1) * NT],
                                    start=(k == 0), stop=(k == DIC - 1))
                            ysl = slice(q * QT + nb2 * NT,
                                        q * QT + (nb2 + 1) * NT)
                            if gi == 0:
                                nc.scalar.activation(y_t[dc][:, ysl], y_ps[:],
                                                     AF.Copy)
                            else:
                                nc.vector.tensor_tensor(
                                    y_t[dc][:, ysl],
                                    y_t[dc][:, ysl].bitcast(F32), y_ps[:],
                                    ALU.add)
            # residual: y += xT (reload from DRAM)
            for blk in ob2:
                for c in range(DC):
                    sl = slice(blk * NT, (blk + 1) * NT)
                    xr = work.tile([128, NT], F32, tag="xr", name="xr",
                                   bufs=2)
                    nc.sync.dma_start(xr[:], xT_d[b, c * 128:(c + 1) * 128, sl])
                    nc.vector.tensor_tensor(y_t[c][:, sl],
                                            y_t[c][:, sl].bitcast(F32), xr[:],
                                            ALU.add)

            # ========================================== phase N2: layernorm2
            norm_stats(y_t, mm_dt,
                       lambda c, sl: y_t[c][:, sl].bitcast(F32), order=ob2)
            yh_t = [big.tile([128, L], mm_dt, tag=f"bufA{c}", name=f"yh{c}")
                    for c in range(DC)]
            norm_apply(yh_t, lambda c, sl: y_t[c][:, sl].bitcast(F32),
                       order=ob2)

            # ========================================== phase M: MLP (2-pass)
            p1w = [load_w(p1_d[k * 128:(k + 1) * 128, :], [128, MLP], f"whg{k}")
                   for k in range(DC)]
            p2w = [load_w(p2_d[k * 128:(k + 1) * 128, :], [128, D],
                          f"wout{k % 6}", bufs=2) for k in range(MLPC)]
            for blk in ob2:
                sl = slice(blk * NT, (blk + 1) * NT)
                ot0 = [work.tile([128, NT], F32, tag=["z", "s", "bb"][dc],
                                 name=f"ot0{dc}", bufs=2) for dc in range(DC)]
                for half in range(2):
                    qs = []
                    for mi in range(6):
                        mc = half * 6 + mi
                        q_ps = psum.tile([128, NT], F32,
                                         tag=("pA" if mi % 2 == 0 else "pB"),
                                         name="q_ps_m")
                        for k in range(DC):
                            nc.tensor.matmul(
                                q_ps[:], p1w[k][:, mc * 128:(mc + 1) * 128],
                                yh_t[k][:, sl],
                                start=(k == 0), stop=(k == DC - 1))
                        qt = work.tile([128, NT], mm_dt, tag=f"hs{mi}",
                                       name=f"q_sb{mi}", bufs=1)
                        nc.scalar.activation(qt[:], q_ps[:], AF.Gelu,
                                             bias=p1b_t[:, mc:mc + 1])
                        qs.append((mc, qt))
                    for dc in range(DC):
                        o_ps = psum.tile([128, NT], F32,
                                         tag=("pC" if dc % 2 == 0 else "pD"),
                                         name="o_ps")
                        for mi, (mc, qt) in enumerate(qs):
                            nc.tensor.matmul(
                                o_ps[:], p2w[mc][:, dc * 128:(dc + 1) * 128],
                                qt[:],
                                start=(mi == 0), stop=(mi == 5))
                        if half == 0:
                            nc.scalar.activation(ot0[dc][:], o_ps[:], AF.Copy)
                        else:
                            ot1 = work.tile([128, NT], F32, tag="sq",
                                            name="ot1", bufs=1)
                            nc.vector.scalar_tensor_tensor(
                                ot1[:], o_ps[:], p2b_t[:, dc:dc + 1],
                                y_t[dc][:, sl].bitcast(F32), ALU.add, ALU.add)
                            oo = work.tile([128, NT], F32, tag="t_ap",
                                           name="oo", bufs=3)
                            nc.vector.tensor_tensor(oo[:], ot0[dc][:], ot1[:],
                                                    ALU.add)
                            nc.sync.dma_start(
                                out_d[b, dc * 128:(dc + 1) * 128, sl], oo[:])

    return nc


# ---------------------------------------------------------------- host side
_NC_CACHE = {}


def _get_nc():
    key = str(MM_DT)
    if key not in _NC_CACHE:
        nc = build_kernel(MM_DT)
        _fix_multiwaits(nc)
        _NC_CACHE[key] = nc
    return _NC_CACHE[key]


def _prep_weights(inp):
    f = np.float32
    dw_w = np.asarray(inp["dw_w"], f)          # [D,1,3,3]
    norm_w = np.asarray(inp["norm_w"], f)
    norm_b = np.asarray(inp["norm_b"], f)
    dw_wf = dw_w[:, 0] * norm_w[:, None, None]     # [D,3,3]
    dw_bf = np.asarray(inp["dw_b"], f) + norm_b * dw_w[:, 0].sum(axis=(1, 2))
    p1_w = np.asarray(inp["p1_w"], f)
    p1f = p1_w * np.asarray(inp["norm2_w"], f)[:, None]
    p1bf = np.asarray(inp["p1_b"], f) + np.asarray(inp["norm2_b"], f) @ p1_w

    # conv diagonal weight blocks: [DC, 128, 9*128]
    diag = np.zeros((DC, 128, 9 * 128), f)
    ar = np.arange(128)
    for c in range(DC):
        for tap in range(9):
            dr, dcc = tap // 3, tap % 3
            diag[c, ar, tap * 128 + ar] = dw_wf[c * 128:(c + 1) * 128, dr, dcc]

    return dict(
        whg1=np.ascontiguousarray(inp["gru1_whg"], f),
        whg2=np.ascontiguousarray(inp["gru2_whg"], f),
        wout1=np.ascontiguousarray(inp["gru1_wout"], f),
        wout2=np.ascontiguousarray(inp["gru2_wout"], f),
        p1=np.ascontiguousarray(p1f, f),
        p2=np.ascontiguousarray(inp["p2_w"], f),
        diag=diag,
        dwb=np.ascontiguousarray(dw_bf.reshape(DC, 128).T, f),
        p1b=np.ascontiguousarray(p1bf.reshape(MLPC, 128).T, f),
        p2b=np.ascontiguousarray(np.asarray(inp["p2_b"], f).reshape(DC, 128).T, f),
    )


def kernel(**inputs):
    x = np.asarray(inputs["x"], np.float32)    # [16, L, D]
    n = x.shape[0]
    w = _prep_weights(inputs)
    nc = _get_nc()

    in_maps = []
    for core in range(NCORES):
        xb = x[core * B:(core + 1) * B]                   # [B, L, D]
        xT = np.ascontiguousarray(xb.transpose(0, 2, 1))  # [B, D, L]
        m = dict(w)
        m["xT"] = xT
        in_maps.append(m)

    res = run_bass_kernel_spmd(nc, in_maps, core_ids=list(range(NCORES)))
    outs = []
    for core in range(NCORES):
        oT = res.results[core]["outT"]                    # [B, D, L]
        outs.append(oT.transpose(0, 2, 1))                # [B, L, D]
    return np.ascontiguousarray(np.concatenate(outs, axis=0), np.float32)

